# revision 1
# baseline (speedup 1.0000x reference)
"""Trainium2 Bass kernel for nn_ConvolutionLayer (FFT conv collapse).

Math: reference computes
    u_fft = rfft(u); ev_fft = rfft(ev)
    p_fft = einsum('bi,kj->bkj', u_fft, ev_fft)      # sums u_fft over i!
    conv  = irfft(p_fft); result = einsum('bkl,k->bl', conv, lam)

The einsum has no shared index, so p_fft[b,k,j] = s_b * ev_fft[k,j] with
s_b = sum_i u_fft[b,i] = u[b,:] @ g   (g = fft(indicator of first L/2+1)).
irfft is R-linear, so with s_b = a_b + i*c_b:
    result[b,:] = a_b * w0 + c_b * w1
    w0 = lam @ ev                       (since irfft(rfft(e)) = e)
    w1 = irfft(i * rfft(w0))            (by linearity over k)
w1 is computed on-device via a 4-step Cooley-Tukey matmul-FFT (64x128),
with the Hermitian symbol (+i / -i / 0) applied in the middle.

Sharding: batch (64) across 8 cores, 8 rows each; the w0/w1 pipeline is
tiny and computed redundantly on every core (no collectives).

Device layouts (per core):
  U    (128p x 512f)   u shard, p = 16*b_loc + t, l = 512*t + f
  EVL  (128p x 2236f)  [EVr | LAMB2]:
        EVr[32s+k, 128t+b]  = ev[k, 128(4t+s)+b]
        LAMB2[32s+k, 60+s]  = LAMB2[32s+k, 124+s] = lam[k]
  xps  (128p x 128f)   [x; x] where x[a,b] = w0[128a+b], built by 16
        accumulating matmuls (lhsT = sliding LAMB2 window)
  FFT: x ->(F64)-> YT ->(*WT)-> ZT ->(F128)-> XT ->(i*sgn)-> X'T
        ->(I128)-> P ->(*Wi)-> Q ->(I64/L, doubled)-> Y2 = [w1; w1]
  final: res_j = a_b * X2 + c_b * Y2 for batch pair b = (2j, 2j+1)
        stacked on the 128 partitions.
"""

import numpy as np

_B, _K, _L = 64, 32, 8192
_NC = 8
_BS = _B // _NC  # 8 batch rows per core
_N1, _N2 = 64, 128  # l = 128*a + b

# ---------------------------------------------------------------- constants


def _build_constants():
    L, N1, N2 = _L, _N1, _N2
    ind = np.zeros(L)
    ind[: L // 2 + 1] = 1.0
    g = np.fft.fft(ind)  # g[n] = sum_{i=0}^{L/2} e^{-2pi i n i/L}

    gU_re = np.tile(g.real.astype(np.float32).reshape(16, 512), (8, 1))
    gU_im = np.tile(g.imag.astype(np.float32).reshape(16, 512), (8, 1))

    # MASK8 (128 x 8): col 2j+i live for partition groups {2j, 2j+1}
    MASK8 = np.zeros((128, 8), np.float32)
    for p in range(128):
        j = (p // 16) // 2
        MASK8[p, 2 * j : 2 * j + 2] = 1.0
    # STK (128 x 128): STK[p, m] = ((p//16) % 2 == m//64)
    STK = np.zeros((128, 128), np.float32)
    for p in range(128):
        STK[p, 64 * ((p // 16) % 2) : 64 * ((p // 16) % 2) + 64] = 1.0

    a_i = np.arange(N1)
    b_i = np.arange(N2)
    F64 = np.exp(-2j * np.pi * np.outer(a_i, a_i) / N1)
    WT = np.exp(-2j * np.pi * np.outer(b_i, a_i) / L)
    F128 = np.exp(-2j * np.pi * np.outer(b_i, b_i) / N2)
    k = a_i[None, :] + N1 * b_i[:, None]  # (128d, 64c)
    sgnT = np.where(
        (k >= 1) & (k <= L // 2 - 1), 1.0, np.where(k > L // 2, -1.0, 0.0)
    )
    I128 = np.exp(+2j * np.pi * np.outer(b_i, b_i) / N2)
    Wi = np.exp(+2j * np.pi * np.outer(a_i, b_i) / L)
    I64s = np.exp(+2j * np.pi * np.outer(a_i, a_i) / N1) / L
    I64re2 = np.hstack([I64s.real, I64s.real])  # (64 x 128) doubled
    I64imN2 = np.hstack([-I64s.imag, -I64s.imag])

    f32 = lambda x: np.ascontiguousarray(np.asarray(x, np.float32))

    p128 = {
        "gU_re": f32(gU_re),
        "gU_im": f32(gU_im),
        "MASK8": f32(MASK8),
        "STK": f32(STK),
        "WT_re": f32(WT.real),
        "WT_im": f32(WT.imag),
        "F128_re": f32(F128.real),
        "F128_im": f32(F128.imag),
        "F128_imN": f32(-F128.imag),
        "sgnT": f32(sgnT),
        "sgnTN": f32(-sgnT),
        "I128_re": f32(I128.real),
        "I128_im": f32(I128.imag),
        "I128_imN": f32(-I128.imag),
    }
    p64 = {
        "F64_re": f32(F64.real),
        "F64_im": f32(F64.imag),
        "Wi_re": f32(Wi.real),
        "Wi_im": f32(Wi.imag),
        "I64_re2": f32(I64re2),
        "I64_imN2": f32(I64imN2),
    }

    def pack(parts):
        off, offs = 0, {}
        for name, arr in parts.items():
            offs[name] = (off, arr.shape[1])
            off += arr.shape[1]
        return np.concatenate(list(parts.values()), axis=1), offs

    CP, cp_off = pack(p128)
    CQ, cq_off = pack(p64)
    return CP, cp_off, CQ, cq_off


_CP, _CP_OFF, _CQ, _CQ_OFF = _build_constants()
_EVL_W = 2048 + 188

# ---------------------------------------------------------------- bass build

_COMPILED = None


def _build_nc():
    import concourse.mybir as mybir
    import concourse.tile as tile
    from concourse import bacc

    f32 = mybir.dt.float32
    Alu = mybir.AluOpType

    nc = bacc.Bacc(None)

    u_d = nc.declare_dram_parameter("u", [128, 512], f32, isOutput=False)
    evl_d = nc.declare_dram_parameter("evl", [128, _EVL_W], f32, isOutput=False)
    cp_d = nc.declare_dram_parameter("cp", list(_CP.shape), f32, isOutput=False)
    cq_d = nc.declare_dram_parameter("cq", list(_CQ.shape), f32, isOutput=False)
    out_d = nc.declare_dram_parameter("out", [_BS, 64, 128], f32, isOutput=True)

    def cp(t, name):
        off, w = _CP_OFF[name]
        return t[:, off : off + w]

    def cq(t, name):
        off, w = _CQ_OFF[name]
        return t[:, off : off + w]

    with tile.TileContext(nc) as tc:
        with (
            tc.tile_pool(name="const", bufs=1) as constp,
            tc.tile_pool(name="sb", bufs=1) as sb,
            tc.tile_pool(name="work", bufs=2) as work,
            tc.tile_pool(name="res", bufs=3) as resp,
            tc.tile_pool(name="psx", bufs=1, space="PSUM") as psx,
            tc.tile_pool(name="psacr", bufs=1, space="PSUM") as psacr,
            tc.tile_pool(name="psfft", bufs=2, space="PSUM") as psfft,
            tc.tile_pool(name="psy", bufs=1, space="PSUM") as psy,
        ):
            EVL = constp.tile([128, _EVL_W], f32)
            U = constp.tile([128, 512], f32)
            CP = constp.tile([128, _CP.shape[1]], f32)
            CQ = constp.tile([64, _CQ.shape[1]], f32)
            nc.sync.dma_start(EVL[:], evl_d[:])
            nc.sync.dma_start(U[:], u_d[:])
            nc.sync.dma_start(CP[:], cp_d[:])
            nc.sync.dma_start(CQ[:], cq_d[:])

            # ---- PE first: xps = [x; x], 16 accumulating matmuls --------
            xps = psx.tile([128, 128], f32)
            for t in range(16):
                nc.tensor.matmul(
                    xps[:],
                    EVL[:, 2048 + 60 - 4 * t : 2048 + 188 - 4 * t],
                    EVL[:, 128 * t : 128 * t + 128],
                    start=(t == 0),
                    stop=(t == 15),
                )

            # ---- DVE: fused per-partition dots  U.g_re, U.g_im ----------
            scratch = sb.tile([128, 512], f32)
            R = sb.tile([128, 2], f32)
            nc.vector.scalar_tensor_tensor(
                scratch[:], U[:], 1.0, cp(CP, "gU_re"),
                op0=Alu.mult, op1=Alu.mult, accum_out=R[:, 0:1],
            )
            nc.vector.scalar_tensor_tensor(
                scratch[:], U[:], 1.0, cp(CP, "gU_im"),
                op0=Alu.mult, op1=Alu.mult, accum_out=R[:, 1:2],
            )
            # R2p (128 x (4j,2i)) = MASK8 * broadcast(R)
            R2p = sb.tile([128, 8], f32)
            nc.vector.tensor_tensor(
                R2p[:].rearrange("p (j i) -> p j i", i=2),
                cp(CP, "MASK8").rearrange("p (j i) -> p j i", i=2),
                R[:].unsqueeze(1).broadcast_to((128, 4, 2)),
                Alu.mult,
            )
            # acrP[m, 2j+i] = ac[2j + m//64, i]  (paired-batch scalars)
            acrP = psacr.tile([128, 8], f32)
            nc.tensor.matmul(acrP[:], cp(CP, "STK"), R2p[:], start=True, stop=True)

            # X2 = [x; x] in SBUF (PE lhsT source + final-stage operand)
            X2 = sb.tile([128, 128], f32)
            nc.vector.tensor_copy(X2[:], xps[:])

            # ---- FFT stage 1: YT[b,c] = sum_a x[a,b] F64[a,c] -----------
            YTre = psfft.tile([128, 64], f32, tag="fftA")
            YTim = psfft.tile([128, 64], f32, tag="fftB")
            nc.tensor.matmul(YTre[:], X2[0:64, :], cq(CQ, "F64_re"), start=True, stop=True)
            nc.tensor.matmul(YTim[:], X2[0:64, :], cq(CQ, "F64_im"), start=True, stop=True)

            # ---- twiddle: ZT = YT * WT (complex) ------------------------
            ZTre = work.tile([128, 64], f32, tag="zt")
            ZTim = work.tile([128, 64], f32, tag="zt2")
            t1 = work.tile([128, 64], f32, tag="t1")
            t2 = work.tile([128, 64], f32, tag="t2")
            nc.vector.tensor_tensor(ZTre[:], YTre[:], cp(CP, "WT_re"), Alu.mult)
            nc.vector.tensor_tensor(t1[:], YTim[:], cp(CP, "WT_im"), Alu.mult)
            nc.vector.tensor_tensor(ZTre[:], ZTre[:], t1[:], Alu.subtract)
            nc.vector.tensor_tensor(ZTim[:], YTre[:], cp(CP, "WT_im"), Alu.mult)
            nc.vector.tensor_tensor(t2[:], YTim[:], cp(CP, "WT_re"), Alu.mult)
            nc.vector.tensor_tensor(ZTim[:], ZTim[:], t2[:], Alu.add)

            # ---- stage 2: XT[d,c] = sum_b F128[b,d] ZT[b,c] -------------
            XTre = psfft.tile([128, 64], f32, tag="fftA")
            XTim = psfft.tile([128, 64], f32, tag="fftB")
            nc.tensor.matmul(XTre[:], cp(CP, "F128_re"), ZTre[:], start=True, stop=False)
            nc.tensor.matmul(XTre[:], cp(CP, "F128_imN"), ZTim[:], start=False, stop=True)
            nc.tensor.matmul(XTim[:], cp(CP, "F128_im"), ZTre[:], start=True, stop=False)
            nc.tensor.matmul(XTim[:], cp(CP, "F128_re"), ZTim[:], start=False, stop=True)

            # ---- symbol: X' = i * sgn * X -------------------------------
            XpTre = work.tile([128, 64], f32, tag="xp")
            XpTim = work.tile([128, 64], f32, tag="xp2")
            nc.vector.tensor_tensor(XpTre[:], XTim[:], cp(CP, "sgnTN"), Alu.mult)
            nc.vector.tensor_tensor(XpTim[:], XTre[:], cp(CP, "sgnT"), Alu.mult)

            # ---- stage 3: P[c,b] = sum_d X'T[d,c] I128[d,b] -------------
            Pre = psfft.tile([64, 128], f32, tag="fftA")
            Pim = psfft.tile([64, 128], f32, tag="fftB")
            nc.tensor.matmul(Pre[:], XpTre[:], cp(CP, "I128_re"), start=True, stop=False)
            nc.tensor.matmul(Pre[:], XpTim[:], cp(CP, "I128_imN"), start=False, stop=True)
            nc.tensor.matmul(Pim[:], XpTre[:], cp(CP, "I128_im"), start=True, stop=False)
            nc.tensor.matmul(Pim[:], XpTim[:], cp(CP, "I128_re"), start=False, stop=True)

            # ---- inverse twiddle: Q = P * Wi (complex) ------------------
            Qre = work.tile([64, 128], f32, tag="q")
            Qim = work.tile([64, 128], f32, tag="q2")
            t3 = work.tile([64, 128], f32, tag="t3")
            t4 = work.tile([64, 128], f32, tag="t4")
            nc.vector.tensor_tensor(Qre[:], Pre[:], cq(CQ, "Wi_re"), Alu.mult)
            nc.vector.tensor_tensor(t3[:], Pim[:], cq(CQ, "Wi_im"), Alu.mult)
            nc.vector.tensor_tensor(Qre[:], Qre[:], t3[:], Alu.subtract)
            nc.vector.tensor_tensor(Qim[:], Pre[:], cq(CQ, "Wi_im"), Alu.mult)
            nc.vector.tensor_tensor(t4[:], Pim[:], cq(CQ, "Wi_re"), Alu.mult)
            nc.vector.tensor_tensor(Qim[:], Qim[:], t4[:], Alu.add)

            # ---- stage 4 (doubled): Y2 = [w1grid; w1grid] ---------------
            Y2 = psy.tile([128, 128], f32)
            nc.tensor.matmul(Y2[:], cq(CQ, "I64_re2"), Qre[:], start=True, stop=False)
            nc.tensor.matmul(Y2[:], cq(CQ, "I64_imN2"), Qim[:], start=False, stop=True)

            # ---- final: paired batches, out DMAs on two queues ----------
            for j in range(_BS // 2):
                tmp = resp.tile([128, 128], f32, tag="tmp")
                nc.vector.tensor_scalar_mul(
                    tmp[:], Y2[:], acrP[:, 2 * j + 1 : 2 * j + 2]
                )
                res = resp.tile([128, 128], f32, tag="res")
                nc.vector.scalar_tensor_tensor(
                    res[:], X2[:], acrP[:, 2 * j : 2 * j + 1], tmp[:],
                    op0=Alu.mult, op1=Alu.add,
                )
                nc.sync.dma_start(out_d[2 * j], res[0:64, :])
                nc.gpsimd.dma_start(out_d[2 * j + 1], res[64:128, :])

    nc.compile()
    return nc


def _get_compiled():
    global _COMPILED
    if _COMPILED is None:
        _COMPILED = _build_nc()
    return _COMPILED


# ---------------------------------------------------------------- entry


def _make_in_maps(u, eigenvectors, eigenvalues):
    u = np.ascontiguousarray(u, np.float32)
    # pure relayout (zero flops): EVr[32s+k, 128t+b] = ev[k, 128(4t+s)+b]
    evr = (
        np.asarray(eigenvectors, np.float32)
        .reshape(_K, 16, 4, 128)
        .transpose(2, 0, 1, 3)
        .reshape(128, 2048)
    )
    lamv = np.asarray(eigenvalues, np.float32)
    lamb2 = np.zeros((128, 188), np.float32)
    for s in range(4):
        lamb2[32 * s : 32 * s + 32, 60 + s] = lamv
        lamb2[32 * s : 32 * s + 32, 124 + s] = lamv
    evl = np.ascontiguousarray(np.hstack([evr, lamb2]))

    in_maps = []
    for c in range(_NC):
        in_maps.append(
            {
                "u": u[c * _BS : (c + 1) * _BS].reshape(128, 512),
                "evl": evl,
                "cp": _CP,
                "cq": _CQ,
            }
        )
    return in_maps, None


def _gather(results):
    outs = [results[c]["out"].reshape(_BS, _L) for c in range(_NC)]
    return np.concatenate(outs, axis=0)


def kernel(u, eigenvectors, eigenvalues):
    from concourse.bass_utils import run_bass_kernel_spmd

    nc = _get_compiled()
    in_maps, _ = _make_in_maps(u, eigenvectors, eigenvalues)
    res = run_bass_kernel_spmd(nc, in_maps, core_ids=list(range(_NC)))
    return _gather(res.results)



# revision 2
# speedup vs baseline: 1.0090x; 1.0090x over previous
"""Trainium2 Bass kernel for nn_ConvolutionLayer (FFT conv collapse), v3.

Math (same collapse as v1): result[b,:] = a_b * w0 + c_b * w1 with
  a_b = sum_{even l} u[b,l] + 4096*u[b,0]      (g_re is 1 on even l, 4097 at 0)
  c_b = sum_{odd l} u[b,l] * (-cot(pi l/L))    (g_im nonzero on odd l only)
  w0  = lam @ ev
  w1  = irfft(i * rfft(w0))  via 64x128 four-step matmul FFT.

v3 design notes (DMA queues run at only ~50-70 GB/s each, so bytes and
arrival order dominate):
  - Everything DMA'd is bf16 except the output; twiddle constants are
    upconverted to fp32 on the Scalar engine in its idle window (DVE
    twiddle reads PSUM fp32 x SBUF fp32).
  - EVL = [LAMB2 | EVr] split into 4 chunk DMAs alternating across the
    sync and gpsimd queues so the 16 lam-reduce matmuls stream with
    chunk arrivals.
  - Each FFT stage writes [re | im] into ONE PSUM tile; a complex
    twiddle is 2 Vector ops (broadcast-AP product against packed
    [Wre|Wim|Wim|-Wre], one strided subtract).  The Hilbert symbol
    i*sgn(j) is one TT against a memset +-1 quadrant mask ([im|re]
    swapped stage-B output); DC/Nyquist corrections vanish through
    I64_im[j1=0,:]=0.  I128 = conj(F128) reuses the F128 tiles.
  - Stage D folds 1/L and the [w1;w1] doubling into its constant.
  - Scalar FIFO: twiddle upconverts, then X2b/X2f (critical), then the
    coefficient-path ops which wait on the late UPK DMA.
"""

import numpy as np
import ml_dtypes

_B, _K, _L = 64, 32, 8192
_NC = 8
_BS = _B // _NC  # 8 batch rows per core

_BF16 = ml_dtypes.bfloat16

# ---------------------------------------------------------------- constants


def _build_constants():
    L = _L
    a_i = np.arange(64)
    n2_i = np.arange(128)
    j1_i = np.arange(64)
    j2_i = np.arange(128)

    F64 = np.exp(-2j * np.pi * np.outer(a_i, j1_i) / 64)      # [a, j1]
    W1 = np.exp(-2j * np.pi * np.outer(n2_i, j1_i) / L)       # [n2, j1]
    F128 = np.exp(-2j * np.pi * np.outer(n2_i, j2_i) / 128)   # [n2, j2] (sym)
    Wi = np.exp(+2j * np.pi * np.outer(j1_i, n2_i) / L)       # [j1, n2]
    I64 = np.exp(+2j * np.pi * np.outer(j1_i, j1_i) / 64) / L
    I64re2 = np.hstack([I64.real, I64.real])                  # (64,128)
    I64imN2 = np.hstack([-I64.imag, -I64.imag])

    STK = np.zeros((128, 128), np.float32)
    for p in range(128):
        STK[p, 64 * ((p // 16) % 2) : 64 * ((p // 16) % 2) + 64] = 1.0

    bf = lambda x: np.ascontiguousarray(np.asarray(x, np.float32)).astype(_BF16)

    # C128 (128p bf16): [F128re | F128im | F128imN | STK]
    c128 = np.hstack([bf(F128.real), bf(F128.imag), bf(-F128.imag), bf(STK)])
    # C64 (64p bf16): [F64re | F64im | I64re2 | I64imN2]
    c64 = np.hstack([bf(F64.real), bf(F64.imag), bf(I64re2), bf(I64imN2)])

    # TWQ (128p bf16): [W1Q 256 | WiQ 512(rows<64)], upconverted on-chip
    # W1Q = [W1re | W1im | W1im | -W1re]; WiQ likewise for Wi.
    w1q = np.hstack([W1.real, W1.imag, W1.imag, -W1.real])
    wiq = np.zeros((128, 512), np.float64)
    wiq[:64, 0:128] = Wi.real
    wiq[:64, 128:256] = Wi.imag
    wiq[:64, 256:384] = Wi.imag
    wiq[:64, 384:512] = -Wi.real
    twq = bf(np.hstack([w1q, wiq]))  # (128, 768)

    # UPK const tail (bf16): [gimO 256 | M4096 1]
    lodd = 512 * (np.arange(128) % 16)[:, None] + 2 * np.arange(256)[None, :] + 1
    gimO = -1.0 / np.tan(np.pi * lodd / L)
    m4096 = np.zeros((128, 1))
    m4096[0::16, 0] = 4096.0
    upack_const = bf(np.hstack([gimO, m4096]))  # (128, 257)
    return c128, c64, twq, upack_const


_C128, _C64, _TWQ, _UPACK_CONST = _build_constants()

_C128_F128RE, _C128_F128IM, _C128_F128IMN, _C128_STK = 0, 128, 256, 384
_C64_F64RE, _C64_F64IM, _C64_I64RE2, _C64_I64IMN2 = 0, 64, 128, 256
_TW_W1Q, _TW_WIQ = 0, 256
# UPK (bf16): [U 512 | gimO 256 | M4096 1]
_UP_U, _UP_GIMO, _UP_M4096 = 0, 512, 768
_UPACK_W = 769
_EVL_W = 188 + 2048  # [LAMB2 188 | EVr 2048]
# chunk boundaries for the 4-way EVL load (cover matmul t ranges)
_EVL_CUTS = [0, 572, 1212, 1724, 2236]  # t0-2 | t3-7 | t8-11 | t12-15

# ---------------------------------------------------------------- bass build

_COMPILED = None


def _build_nc():
    import concourse.mybir as mybir
    import concourse.tile as tile
    from concourse import bacc

    f32 = mybir.dt.float32
    bf16 = mybir.dt.bfloat16
    Alu = mybir.AluOpType
    Act = mybir.ActivationFunctionType

    nc = bacc.Bacc(None)

    evl_d = nc.declare_dram_parameter("evl", [128, _EVL_W], bf16, isOutput=False)
    upack_d = nc.declare_dram_parameter("upack", [128, _UPACK_W], bf16, isOutput=False)
    twq_d = nc.declare_dram_parameter("twq", [128, 768], bf16, isOutput=False)
    c128_d = nc.declare_dram_parameter("c128", [128, 512], bf16, isOutput=False)
    c64_d = nc.declare_dram_parameter("c64", [64, 384], bf16, isOutput=False)
    out_d = nc.declare_dram_parameter("out", [_BS, 64, 128], f32, isOutput=True)

    with tile.TileContext(nc) as tc:
        with (
            tc.tile_pool(name="const", bufs=1) as constp,
            tc.tile_pool(name="sb", bufs=1) as sb,
            tc.tile_pool(name="psx", bufs=1, space="PSUM") as psx,
            tc.tile_pool(name="psacr", bufs=1, space="PSUM") as psacr,
            tc.tile_pool(name="ps1", bufs=1, space="PSUM") as ps1,
            tc.tile_pool(name="ps2", bufs=1, space="PSUM") as ps2,
        ):
            EVL = constp.tile([128, _EVL_W], bf16)
            UPK = constp.tile([128, _UPACK_W], bf16)
            TWQ = constp.tile([128, 768], bf16)
            C128 = constp.tile([128, 512], bf16)
            C64 = constp.tile([64, 384], bf16)

            # memset-built symbol mask (32-partition aligned quadrants)
            BSF = sb.tile([128, 128], f32)
            MASK8 = sb.tile([128, 8], f32)
            nc.vector.memset(BSF[0:64, 0:64], -1.0)
            nc.vector.memset(BSF[0:64, 64:128], 1.0)
            nc.vector.memset(BSF[64:128, 0:64], 1.0)
            nc.vector.memset(BSF[64:128, 64:128], -1.0)
            nc.vector.memset(MASK8[:], 0.0)
            for j in range(4):
                nc.vector.memset(MASK8[32 * j : 32 * j + 32, 2 * j : 2 * j + 2], 1.0)

            # input DMAs: EVL chunks alternate sync/gpsimd; consts behind
            c = _EVL_CUTS
            nc.sync.dma_start(EVL[:, c[0] : c[1]], evl_d[:, c[0] : c[1]])
            nc.gpsimd.dma_start(EVL[:, c[1] : c[2]], evl_d[:, c[1] : c[2]])
            nc.sync.dma_start(EVL[:, c[2] : c[3]], evl_d[:, c[2] : c[3]])
            nc.gpsimd.dma_start(EVL[:, c[3] : c[4]], evl_d[:, c[3] : c[4]])
            nc.scalar.dma_start(TWQ[:], twq_d[:])
            nc.gpsimd.dma_start(C64[:], c64_d[:])
            nc.scalar.dma_start(C128[:], c128_d[:])
            nc.gpsimd.dma_start(UPK[:], upack_d[:])

            # ---- PE: xps = [x; x], 16 accumulating bf16 matmuls ---------
            xps = psx.tile([128, 128], f32)
            for t in range(16):
                nc.tensor.matmul(
                    xps[:],
                    EVL[:, 60 - 4 * t : 188 - 4 * t],
                    EVL[:, 188 + 128 * t : 188 + 128 * t + 128],
                    start=(t == 0),
                    stop=(t == 15),
                )

            # ---- Scalar: upconvert twiddle packs to fp32 ----------------
            W1Qf = sb.tile([128, 256], f32)
            WiQf = sb.tile([64, 512], f32)
            nc.scalar.activation(W1Qf[:], TWQ[:, _TW_W1Q : _TW_W1Q + 256], Act.Copy)
            nc.scalar.activation(WiQf[:], TWQ[0:64, _TW_WIQ : _TW_WIQ + 512], Act.Copy)

            # ---- copies out of xps PSUM (Scalar, critical first) --------
            X2b = sb.tile([64, 128], bf16)     # x grid for FFT (bf16)
            X2f = sb.tile([128, 128], f32)     # [x; x] for final stage
            nc.scalar.activation(X2b[:], xps[0:64, :], Act.Copy)
            nc.scalar.activation(X2f[:], xps[:], Act.Copy)

            # ---- stage A: S2cat = [S2re | S2im] (128 x 128 PSUM) --------
            S2cat = ps1.tile([128, 128], f32, tag="s2")
            nc.tensor.matmul(S2cat[:, 0:64], X2b[:], C64[:, _C64_F64RE : _C64_F64RE + 64], start=True, stop=True)
            nc.tensor.matmul(S2cat[:, 64:128], X2b[:], C64[:, _C64_F64IM : _C64_F64IM + 64], start=True, stop=True)

            # ---- coefficient dots (bf16 in, fp32 accum) -----------------
            scrA = sb.tile([128, 256], f32)
            scrC = sb.tile([128, 256], f32)
            scrF = sb.tile([128, 1], f32)
            Rraw = sb.tile([128, 1], f32)
            R = sb.tile([128, 2], f32)
            nc.scalar.activation(
                scrA[:], UPK[:, _UP_U : _UP_U + 256], Act.Copy, accum_out=Rraw[:]
            )
            nc.vector.tensor_tensor(
                scrF[:], UPK[:, _UP_U : _UP_U + 1],
                UPK[:, _UP_M4096 : _UP_M4096 + 1], Alu.mult,
            )
            nc.vector.tensor_tensor(R[:, 0:1], Rraw[:], scrF[:], Alu.add)
            nc.vector.scalar_tensor_tensor(
                scrC[:], UPK[:, _UP_U + 256 : _UP_U + 512], 1.0,
                UPK[:, _UP_GIMO : _UP_GIMO + 256],
                op0=Alu.mult, op1=Alu.mult, accum_out=R[:, 1:2],
            )
            R2p = sb.tile([128, 8], bf16)
            nc.vector.tensor_tensor(
                R2p[:].rearrange("p (j i) -> p j i", i=2),
                MASK8[:].rearrange("p (j i) -> p j i", i=2),
                R[:].unsqueeze(1).broadcast_to((128, 4, 2)),
                Alu.mult,
            )
            acrP = psacr.tile([128, 8], f32)
            nc.tensor.matmul(acrP[:], C128[:, _C128_STK : _C128_STK + 128], R2p[:], start=True, stop=True)
            acrS = sb.tile([128, 8], f32)
            nc.scalar.activation(acrS[:], acrP[:], Act.Copy)

            # ---- twiddle 1 (2 V ops): Zcat = [Zre | Zim] bf16 -----------
            prodAB = sb.tile([128, 256], f32)
            nc.vector.tensor_tensor(
                prodAB[:].rearrange("p (r f) -> p r f", r=2),
                S2cat[:].unsqueeze(1).broadcast_to((128, 2, 128)),
                W1Qf[:].rearrange("p (r f) -> p r f", r=2),
                Alu.mult,
            )
            Zcat = sb.tile([128, 128], bf16)
            pv = prodAB[:].rearrange("p (r f) -> p r f", f=64)
            nc.vector.tensor_tensor(
                Zcat[:].rearrange("p (r f) -> p r f", f=64),
                pv[:, 0:4:2, :], pv[:, 1:4:2, :], Alu.subtract,
            )

            # ---- stage B: Xswap = [Xgim | Xgre] -------------------------
            F128re = C128[:, _C128_F128RE : _C128_F128RE + 128]
            F128im = C128[:, _C128_F128IM : _C128_F128IM + 128]
            F128imN = C128[:, _C128_F128IMN : _C128_F128IMN + 128]
            Zre = Zcat[:, 0:64]
            Zim = Zcat[:, 64:128]
            Xswap = ps2.tile([128, 128], f32, tag="xg")
            nc.tensor.matmul(Xswap[:, 0:64], F128im, Zre, start=True, stop=False)
            nc.tensor.matmul(Xswap[:, 0:64], F128re, Zim, start=False, stop=True)
            nc.tensor.matmul(Xswap[:, 64:128], F128re, Zre, start=True, stop=False)
            nc.tensor.matmul(Xswap[:, 64:128], F128imN, Zim, start=False, stop=True)

            # ---- symbol (1 V op): Xpcat = BSF * Xswap -------------------
            Xpcat = sb.tile([128, 128], bf16)
            nc.vector.tensor_tensor(Xpcat[:], Xswap[:], BSF[:], Alu.mult)

            # ---- stage C: U1cat = [U1re | U1im] (64 x 256 PSUM) ---------
            Xpre = Xpcat[:, 0:64]
            Xpim = Xpcat[:, 64:128]
            U1cat = ps1.tile([64, 256], f32, tag="s2")
            nc.tensor.matmul(U1cat[:, 0:128], Xpre, F128re, start=True, stop=False)
            nc.tensor.matmul(U1cat[:, 0:128], Xpim, F128im, start=False, stop=True)
            nc.tensor.matmul(U1cat[:, 128:256], Xpre, F128imN, start=True, stop=False)
            nc.tensor.matmul(U1cat[:, 128:256], Xpim, F128re, start=False, stop=True)

            # res_j = a_j*X2f base terms in Scalar idle window
            res = sb.tile([128, 512], f32)
            for j in range(4):
                nc.scalar.activation(
                    res[:, 128 * j : 128 * j + 128], X2f[:], Act.Copy,
                    scale=acrS[:, 2 * j : 2 * j + 1],
                )

            # ---- twiddle 2 (2 V ops): Vcat = [Vre | Vim] bf16 -----------
            prod2 = sb.tile([64, 512], f32)
            nc.vector.tensor_tensor(
                prod2[:].rearrange("p (r f) -> p r f", r=2),
                U1cat[:].unsqueeze(1).broadcast_to((64, 2, 256)),
                WiQf[:].rearrange("p (r f) -> p r f", r=2),
                Alu.mult,
            )
            Vcat = sb.tile([64, 256], bf16)
            p2 = prod2[:].rearrange("p (r f) -> p r f", f=128)
            nc.vector.tensor_tensor(
                Vcat[:].rearrange("p (r f) -> p r f", f=128),
                p2[:, 0:4:2, :], p2[:, 1:4:2, :], Alu.subtract,
            )

            # ---- stage D: Y2 = [w1; w1] ---------------------------------
            Y2 = ps2.tile([128, 128], f32, tag="xg")
            nc.tensor.matmul(Y2[:], C64[:, _C64_I64RE2 : _C64_I64RE2 + 128], Vcat[:, 0:128], start=True, stop=False)
            nc.tensor.matmul(Y2[:], C64[:, _C64_I64IMN2 : _C64_I64IMN2 + 128], Vcat[:, 128:256], start=False, stop=True)

            # ---- final: res_j += c_j*Y2 on V; DMA per pair --------------
            outeng = [nc.sync, nc.scalar, nc.sync, nc.scalar]
            for j in range(4):
                nc.vector.scalar_tensor_tensor(
                    res[:, 128 * j : 128 * j + 128], Y2[:],
                    acrS[:, 2 * j + 1 : 2 * j + 2],
                    res[:, 128 * j : 128 * j + 128],
                    op0=Alu.mult, op1=Alu.add,
                )
                outeng[j].dma_start(
                    out_d[2 * j : 2 * j + 2], res[:, 128 * j : 128 * j + 128]
                )

    nc.compile()
    return nc


def _get_compiled():
    global _COMPILED
    if _COMPILED is None:
        _COMPILED = _build_nc()
    return _COMPILED


# ---------------------------------------------------------------- entry


def _make_in_maps(u, eigenvectors, eigenvalues):
    u = np.ascontiguousarray(u, np.float32)
    # EVr[32s+k, 128t+b] = ev[k, 128(4t+s)+b], bf16
    evr = (
        np.asarray(eigenvectors, np.float32)
        .reshape(_K, 16, 4, 128)
        .transpose(2, 0, 1, 3)
        .reshape(128, 2048)
        .astype(_BF16)
    )
    lamv = np.asarray(eigenvalues, np.float32)
    lamb2 = np.zeros((128, 188), np.float32)
    for s in range(4):
        lamb2[32 * s : 32 * s + 32, 60 + s] = lamv
        lamb2[32 * s : 32 * s + 32, 124 + s] = lamv
    evl = np.ascontiguousarray(np.hstack([lamb2.astype(_BF16), evr]))

    in_maps = []
    for c in range(_NC):
        uc = u[c * _BS : (c + 1) * _BS].reshape(128, 512)  # p = 16b+t
        U = np.empty((128, 512), np.float32)
        U[:, 0:256] = uc[:, 0::2]
        U[:, 256:512] = uc[:, 1::2]
        upack = np.ascontiguousarray(
            np.hstack([U.astype(_BF16), _UPACK_CONST])
        )
        in_maps.append(
            {
                "evl": evl,
                "upack": upack,
                "twq": _TWQ,
                "c128": _C128,
                "c64": _C64,
            }
        )
    return in_maps, None


def _gather(results):
    outs = [results[c]["out"].reshape(_BS, _L) for c in range(_NC)]
    return np.concatenate(outs, axis=0)


def kernel(u, eigenvectors, eigenvalues):
    from concourse.bass_utils import run_bass_kernel_spmd

    nc = _get_compiled()
    in_maps, _ = _make_in_maps(u, eigenvectors, eigenvalues)
    res = run_bass_kernel_spmd(nc, in_maps, core_ids=list(range(_NC)))
    return _gather(res.results)
